# revision 8
# baseline (speedup 1.0000x reference)
"""Paged-attention decode (GQA) on 8 Trainium2 NeuronCores.

Sharding: tensor-parallel over KV heads — core h owns kv-head h for all 16
sequences. The per-core cache slice [256, 256, 128] is contiguous, every core
runs an identical instruction stream (same per-seq block counts), so the SPMD
program is perfectly uniform with zero padding waste.

Per core, per sequence (nb = ceil(cl/256) blocks):
  stream K blocks (HWDGE, 128KB contiguous) -> PE transpose -> bf16 K^T
  -> QK^T matmul (bf16, q^T stationary) -> +boundary mask -> ACT exp
  (fused *SCALE, accum_out gives softmax denominator) -> w bf16
  -> PE transpose w -> PV matmul (bf16, w^T stationary, V natural layout)
  -> scale by 1/denom -> out.

The current-step K/V scatter (slot_mapping) is applied host-side while
staging the per-core cache slices; q is pre-transposed/cast host-side.
"""

import sys

sys.path.insert(0, "/opt/trn_rl_repo")

import numpy as np
from ml_dtypes import bfloat16

import concourse.bass as bass
import concourse.bacc as bacc
import concourse.mybir as mybir
from concourse import bass_utils
from concourse.tile import TileContext
from concourse.masks import make_identity

NUM_BLOCKS = 256
BLOCK_SIZE = 256
BATCH = 16
MAX_BLOCKS = 8
NUM_HEADS = 32
NUM_KV_HEADS = 8
HEAD_DIM = 128
G = NUM_HEADS // NUM_KV_HEADS  # 4
SCALE = float(1.0 / np.sqrt(HEAD_DIM))
N_CORES = 8
P = 128

_nc_cache: dict = {}


def _build_nc(NB, BT):
    """Build the (per-core uniform) Bass program. NB[b] = block count of seq b,
    BT[b][i] = block id (compile-time DMA addresses)."""
    f32 = mybir.dt.float32
    bf16 = mybir.dt.bfloat16
    Exp = mybir.ActivationFunctionType.Exp
    Copy = mybir.ActivationFunctionType.Copy

    nc = bacc.Bacc(None, target_bir_lowering=False)
    kc = nc.dram_tensor("kc", [NUM_BLOCKS, BLOCK_SIZE, HEAD_DIM], f32, kind="ExternalInput")
    vc = nc.dram_tensor("vc", [NUM_BLOCKS, BLOCK_SIZE, HEAD_DIM], f32, kind="ExternalInput")
    qt = nc.dram_tensor("qt", [P, BATCH * G], bf16, kind="ExternalInput")
    mk = nc.dram_tensor("mask", [G, BATCH * BLOCK_SIZE], f32, kind="ExternalInput")
    out = nc.dram_tensor("out", [BATCH, G, HEAD_DIM], f32, kind="ExternalOutput")

    with TileContext(nc) as tc:
        with (
            tc.tile_pool(name="const", bufs=1) as constp,
            tc.tile_pool(name="kv", bufs=8) as kvp,
            tc.tile_pool(name="vb", bufs=2 * MAX_BLOCKS) as vbp,
            tc.tile_pool(name="kt", bufs=6) as ktp,
            tc.tile_pool(name="w", bufs=2) as wp,
            tc.tile_pool(name="sm", bufs=3) as smp,
            tc.tile_pool(name="pss", bufs=2, space="PSUM") as pss,
            tc.tile_pool(name="psk", bufs=2, space="PSUM") as psk,
            tc.tile_pool(name="pso", bufs=2, space="PSUM") as pso,
        ):
            idf = constp.tile([P, P], f32, tag="idf")
            make_identity(nc, idf[:])
            idb = constp.tile([P, P], bf16, tag="idb")
            make_identity(nc, idb[:])
            qt_sb = constp.tile([P, BATCH * G], bf16, tag="qt")
            nc.sync.dma_start(out=qt_sb[:], in_=qt[:, :])
            mk_sb = constp.tile([G, BATCH * BLOCK_SIZE], f32, tag="mk")
            nc.sync.dma_start(out=mk_sb[:], in_=mk[:, :])

            for b in range(BATCH):
                nb = NB[b]
                w_t = wp.tile([G, MAX_BLOCKS * BLOCK_SIZE], bf16, tag="w")
                part = smp.tile([G, MAX_BLOCKS], f32, tag="part")
                vb_list = []
                for i in range(nb):
                    blk = BT[b][i]
                    # [256, 128] -> [128 (tok//2), (tok%2, d)]; 1KB/partition chunks
                    k_src = kc[blk].rearrange("(p two) d -> p (two d)", two=2)
                    v_src = vc[blk].rearrange("(p two) d -> p (two d)", two=2)
                    k_t = kvp.tile([P, 2 * HEAD_DIM], f32, tag="k")
                    nc.sync.dma_start(out=k_t[:], in_=k_src)
                    v_t = kvp.tile([P, 2 * HEAD_DIM], f32, tag="v")
                    nc.scalar.dma_start(out=v_t[:], in_=v_src)
                    v_b = vbp.tile([P, 2 * HEAD_DIM], bf16, tag="vb")
                    nc.vector.tensor_copy(out=v_b[:], in_=v_t[:])
                    vb_list.append(v_b)

                    kt_ps = psk.tile([P, 2 * HEAD_DIM], f32, tag="ktps")
                    nc.tensor.transpose(out=kt_ps[:, 0:P], in_=k_t[:, 0:P], identity=idf[:])
                    nc.tensor.transpose(out=kt_ps[:, P : 2 * P], in_=k_t[:, P : 2 * P], identity=idf[:])
                    kt_t = ktp.tile([P, 2 * HEAD_DIM], bf16, tag="kt")
                    nc.vector.tensor_copy(out=kt_t[:], in_=kt_ps[:])

                    s_ps = pss.tile([G, BLOCK_SIZE], f32, tag="sps")
                    nc.tensor.matmul(
                        out=s_ps[:], lhsT=qt_sb[:, G * b : G * (b + 1)], rhs=kt_t[:],
                        start=True, stop=True,
                    )
                    if i == nb - 1:
                        nc.vector.tensor_tensor(
                            out=s_ps[:], in0=s_ps[:],
                            in1=mk_sb[:, BLOCK_SIZE * b : BLOCK_SIZE * (b + 1)],
                            op=mybir.AluOpType.add,
                        )
                    nc.scalar.activation(
                        out=w_t[:, BLOCK_SIZE * i : BLOCK_SIZE * (i + 1)], in_=s_ps[:],
                        func=Exp, scale=SCALE, accum_out=part[:, i : i + 1],
                    )

                wt_ps = pso.tile([P, 2 * G * MAX_BLOCKS], bf16, tag="wtps")
                for i in range(nb):
                    nc.tensor.transpose(
                        out=wt_ps[:, 2 * G * i : 2 * G * i + G],
                        in_=w_t[:, BLOCK_SIZE * i : BLOCK_SIZE * i + P],
                        identity=idb[:G, :G],
                    )
                    nc.tensor.transpose(
                        out=wt_ps[:, 2 * G * i + G : 2 * G * (i + 1)],
                        in_=w_t[:, BLOCK_SIZE * i + P : BLOCK_SIZE * (i + 1)],
                        identity=idb[:G, :G],
                    )
                wt_t = smp.tile([P, 2 * G * MAX_BLOCKS], bf16, tag="wt")
                nc.vector.tensor_copy(out=wt_t[:, : 2 * G * nb], in_=wt_ps[:, : 2 * G * nb])

                o_ps = pso.tile([G, HEAD_DIM], f32, tag="ops")
                for i in range(nb):
                    nc.tensor.matmul(
                        out=o_ps[:], lhsT=wt_t[:, 2 * G * i : 2 * G * i + G],
                        rhs=vb_list[i][:, 0:HEAD_DIM], start=(i == 0), stop=False,
                    )
                    nc.tensor.matmul(
                        out=o_ps[:], lhsT=wt_t[:, 2 * G * i + G : 2 * G * (i + 1)],
                        rhs=vb_list[i][:, HEAD_DIM : 2 * HEAD_DIM],
                        start=False, stop=(i == nb - 1),
                    )

                den = smp.tile([G, 1], f32, tag="den")
                nc.vector.reduce_sum(out=den[:], in_=part[:, :nb], axis=mybir.AxisListType.X)
                rec = smp.tile([G, 1], f32, tag="rec")
                nc.vector.reciprocal(out=rec[:], in_=den[:])
                o_sb = smp.tile([G, HEAD_DIM], f32, tag="osb")
                nc.scalar.activation(out=o_sb[:], in_=o_ps[:], func=Copy, scale=rec[:, 0:1])
                nc.sync.dma_start(out=out[b], in_=o_sb[:])
    nc.compile()
    return nc


def kernel(q, k, v, k_cache, v_cache, block_tables, context_lens, slot_mapping):
    q = np.asarray(q, dtype=np.float32)
    k = np.asarray(k, dtype=np.float32)
    v = np.asarray(v, dtype=np.float32)
    kc = np.array(k_cache, dtype=np.float32).reshape(-1, NUM_KV_HEADS, HEAD_DIM)
    vcf = np.array(v_cache, dtype=np.float32).reshape(-1, NUM_KV_HEADS, HEAD_DIM)
    bt = np.clip(np.asarray(block_tables, dtype=np.int64), 0, NUM_BLOCKS - 1)
    cl = np.asarray(context_lens, dtype=np.int64)
    sm = np.asarray(slot_mapping, dtype=np.int64)

    # current-step K/V scatter (reference._store_kv), host-side while staging
    valid = sm >= 0
    kc[sm[valid]] = k[valid]
    vcf[sm[valid]] = v[valid]
    kc = kc.reshape(NUM_BLOCKS, BLOCK_SIZE, NUM_KV_HEADS, HEAD_DIM)
    vcf = vcf.reshape(NUM_BLOCKS, BLOCK_SIZE, NUM_KV_HEADS, HEAD_DIM)

    NB = np.maximum(1, -(-cl // BLOCK_SIZE)).astype(np.int64)

    # additive boundary mask, permuted token order (col c of a block holds
    # token 2*(c%128) + c//128), replicated across the G query heads
    c = np.arange(BLOCK_SIZE)
    tok = 2 * (c % P) + (c // P)
    mask = np.zeros((BATCH, G, BLOCK_SIZE), dtype=np.float32)
    for b in range(BATCH):
        cl_loc = cl[b] - BLOCK_SIZE * (NB[b] - 1)
        mask[b, :, :] = np.where(tok < cl_loc, 0.0, -1e9)[None, :]
    mask = np.ascontiguousarray(mask.transpose(1, 0, 2).reshape(G, BATCH * BLOCK_SIZE))

    key = (bt.tobytes(), NB.tobytes(), cl.tobytes())
    nc = _nc_cache.get(key)
    if nc is None:
        nc = _build_nc([int(x) for x in NB], [[int(x) for x in row] for row in bt])
        _nc_cache.clear()
        _nc_cache[key] = nc

    qg = q.reshape(BATCH, NUM_KV_HEADS, G, HEAD_DIM)
    in_maps = []
    for h in range(N_CORES):
        qt_h = np.ascontiguousarray(
            qg[:, h].transpose(2, 0, 1).reshape(P, BATCH * G)
        ).astype(bfloat16)
        in_maps.append(
            {
                "kc": np.ascontiguousarray(kc[:, :, h, :]),
                "vc": np.ascontiguousarray(vcf[:, :, h, :]),
                "qt": qt_h,
                "mask": mask,
            }
        )

    global _last_in_maps
    _last_in_maps = in_maps
    res = bass_utils.run_bass_kernel_spmd(nc, in_maps, core_ids=list(range(N_CORES)))
    outs = np.stack([res.results[h]["out"] for h in range(N_CORES)], axis=1)
    return np.ascontiguousarray(outs.reshape(BATCH, NUM_HEADS, HEAD_DIM)).astype(np.float32)


# revision 15
# speedup vs baseline: 1.4862x; 1.4862x over previous
"""Paged-attention decode (GQA) on 8 Trainium2 NeuronCores.

Sharding: tensor-parallel over KV heads — core h owns kv-head h for all 16
sequences. The per-core cache slice [256, 256, 128] is contiguous, every core
runs an identical instruction stream (same per-seq block counts), so the SPMD
program is perfectly uniform with zero padding waste.

Per core, per sequence (nb = ceil(cl/256) blocks):
  stream K blocks (HWDGE, 128KB contiguous) -> PE transpose -> bf16 K^T
  -> QK^T matmul (bf16, q^T stationary) -> +boundary mask -> ACT exp
  (fused *SCALE, accum_out gives softmax denominator) -> w bf16
  -> PE transpose w -> PV matmul (bf16, w^T stationary, V natural layout)
  -> scale by 1/denom -> out.

The current-step K/V scatter (slot_mapping) is applied host-side while
staging the per-core cache slices; q is pre-transposed/cast host-side.
"""

import sys

sys.path.insert(0, "/opt/trn_rl_repo")

import numpy as np
from ml_dtypes import bfloat16

import concourse.bass as bass
import concourse.bacc as bacc
import concourse.mybir as mybir
from concourse import bass_utils
from concourse.tile import TileContext
from concourse.masks import make_identity

NUM_BLOCKS = 256
BLOCK_SIZE = 256
BATCH = 16
MAX_BLOCKS = 8
NUM_HEADS = 32
NUM_KV_HEADS = 8
HEAD_DIM = 128
G = NUM_HEADS // NUM_KV_HEADS  # 4
SCALE = float(1.0 / np.sqrt(HEAD_DIM))
N_CORES = 8
P = 128

_nc_cache: dict = {}
PAIR_STEP = 2


def _build_nc(NB, BT):
    """Build the (per-core uniform) Bass program. NB[b] = block count of seq b,
    BT[b][i] = block id (compile-time DMA addresses)."""
    f32 = mybir.dt.float32
    bf16 = mybir.dt.bfloat16
    Exp = mybir.ActivationFunctionType.Exp
    Copy = mybir.ActivationFunctionType.Copy

    nc = bacc.Bacc(None, target_bir_lowering=False)
    kc = nc.dram_tensor("kc", [NUM_BLOCKS, BLOCK_SIZE, HEAD_DIM], f32, kind="ExternalInput")
    vc = nc.dram_tensor("vc", [NUM_BLOCKS, BLOCK_SIZE, HEAD_DIM], f32, kind="ExternalInput")
    qt = nc.dram_tensor("qt", [P, BATCH * G], bf16, kind="ExternalInput")
    mk = nc.dram_tensor("mask", [G, BATCH * BLOCK_SIZE], f32, kind="ExternalInput")
    out = nc.dram_tensor("out", [BATCH, G, HEAD_DIM], f32, kind="ExternalOutput")

    # [256 blk, 256 tok, 128 d] -> [blk, p=tok//2, (tok%2, d)]
    kc3 = kc.rearrange("nb (p two) d -> nb p (two d)", two=2)
    vc3 = vc.rearrange("nb (p two) d -> nb p (two d)", two=2)

    def chunk_src(t3, blks):
        if len(blks) == 1:
            return t3[blks[0]]
        lo, hi = min(blks), max(blks)
        return t3[lo : hi + 1 : hi - lo].transpose([1, 0, 2])

    dma_ring = [nc.sync, nc.scalar]

    with TileContext(nc) as tc:
        with (
            tc.tile_pool(name="const", bufs=1) as constp,
            tc.tile_pool(name="kv", bufs=4) as kvp,
            tc.tile_pool(name="vb", bufs=7) as vbp,
            tc.tile_pool(name="kt", bufs=4) as ktp,
            tc.tile_pool(name="w", bufs=2) as wp,
            tc.tile_pool(name="sm", bufs=3) as smp,
            tc.tile_pool(name="pss", bufs=2, space="PSUM") as pss,
            tc.tile_pool(name="psk", bufs=2, space="PSUM") as psk,
            tc.tile_pool(name="pso", bufs=2, space="PSUM") as pso,
        ):
            idf = constp.tile([P, P], f32, tag="idf")
            make_identity(nc, idf[:])
            idb = constp.tile([P, P], bf16, tag="idb")
            make_identity(nc, idb[:])
            qt_sb = constp.tile([P, BATCH * G], bf16, tag="qt")
            nc.sync.dma_start(out=qt_sb[:], in_=qt[:, :])
            mk_sb = constp.tile([G, BATCH * BLOCK_SIZE], f32, tag="mk")
            nc.sync.dma_start(out=mk_sb[:], in_=mk[:, :])

            ndma = 0
            for b in range(BATCH):
                nb = NB[b]
                # chunks of 1-2 blocks; within a pair, blocks sorted ascending
                # (positive DMA stride). chunk_pos[i] = (chunk, half) of BT[b][i].
                chunks = []
                chunk_pos = {}
                for i in range(0, nb, PAIR_STEP):
                    grp = BT[b][i : min(i + PAIR_STEP, nb)]
                    if len(grp) == 2 and grp[0] != grp[1]:
                        blks = sorted(grp)
                        for orig in range(i, i + 2):
                            chunk_pos[orig] = (len(chunks), blks.index(BT[b][orig]))
                        chunks.append(blks)
                    else:
                        for orig in range(i, min(i + PAIR_STEP, nb)):
                            chunk_pos[orig] = (len(chunks), 0)
                            chunks.append([BT[b][orig]])

                w_t = wp.tile([G, MAX_BLOCKS * BLOCK_SIZE], bf16, tag="w")
                part = smp.tile([G, MAX_BLOCKS], f32, tag="part")
                vb_list = []
                base = 0
                for ci, blks in enumerate(chunks):
                    W = 2 * HEAD_DIM * len(blks)
                    k_t = kvp.tile([P, 2 * 2 * HEAD_DIM], f32, tag="k")
                    eng = dma_ring[ndma % 2]
                    ndma += 1
                    kdst = k_t[:, :W]
                    if len(blks) == 2:
                        kdst = kdst.rearrange("p (c td) -> p c td", c=2)
                    eng.dma_start(out=kdst, in_=chunk_src(kc3, blks))
                    # V: SWDGE cast-DMA (f32 HBM -> bf16 SBUF), gpsimd ring
                    v_b = vbp.tile([P, 2 * 2 * HEAD_DIM], bf16, tag="vb")
                    vdst = v_b[:, :W]
                    if len(blks) == 2:
                        vdst = vdst.rearrange("p (c td) -> p c td", c=2)
                    nc.gpsimd.dma_start(out=vdst, in_=chunk_src(vc3, blks))
                    vb_list.append(v_b)

                    kt_ps = psk.tile([P, 2 * 2 * HEAD_DIM], f32, tag="ktps")
                    for s in range(W // P):
                        nc.tensor.transpose(
                            out=kt_ps[:, P * s : P * (s + 1)],
                            in_=k_t[:, P * s : P * (s + 1)], identity=idf[:],
                        )
                    kt_t = ktp.tile([P, 2 * 2 * HEAD_DIM], bf16, tag="kt")
                    nc.vector.tensor_copy(out=kt_t[:, :W], in_=kt_ps[:, :W])

                    s_ps = pss.tile([G, 2 * BLOCK_SIZE], f32, tag="sps")
                    nc.tensor.matmul(
                        out=s_ps[:, :W], lhsT=qt_sb[:, G * b : G * (b + 1)],
                        rhs=kt_t[:, :W], start=True, stop=True,
                    )
                    bci, bh = chunk_pos[nb - 1]
                    if bci == ci:  # boundary block lives in this chunk
                        off = BLOCK_SIZE * bh
                        nc.vector.tensor_tensor(
                            out=s_ps[:, off : off + BLOCK_SIZE],
                            in0=s_ps[:, off : off + BLOCK_SIZE],
                            in1=mk_sb[:, BLOCK_SIZE * b : BLOCK_SIZE * (b + 1)],
                            op=mybir.AluOpType.add,
                        )
                    nc.scalar.activation(
                        out=w_t[:, base : base + W], in_=s_ps[:, :W],
                        func=Exp, scale=SCALE, accum_out=part[:, ci : ci + 1],
                    )
                    base += W

                nchunk = len(chunks)
                wt_ps = pso.tile([P, 2 * G * MAX_BLOCKS], bf16, tag="wtps")
                for j in range(2 * nb):
                    nc.tensor.transpose(
                        out=wt_ps[:, G * j : G * (j + 1)],
                        in_=w_t[:, P * j : P * (j + 1)],
                        identity=idb[:G, :G],
                    )
                wt_t = smp.tile([P, 2 * G * MAX_BLOCKS], bf16, tag="wt")
                nc.vector.tensor_copy(out=wt_t[:, : 2 * G * nb], in_=wt_ps[:, : 2 * G * nb])

                o_ps = pso.tile([G, HEAD_DIM], f32, tag="ops")
                j = 0
                for ci, blks in enumerate(chunks):
                    for s in range(2 * len(blks)):
                        nc.tensor.matmul(
                            out=o_ps[:], lhsT=wt_t[:, G * j : G * (j + 1)],
                            rhs=vb_list[ci][:, P * s : P * (s + 1)],
                            start=(j == 0), stop=(j == 2 * nb - 1),
                        )
                        j += 1

                den = smp.tile([G, 1], f32, tag="den")
                nc.vector.reduce_sum(out=den[:], in_=part[:, :nchunk], axis=mybir.AxisListType.X)
                rec = smp.tile([G, 1], f32, tag="rec")
                nc.vector.reciprocal(out=rec[:], in_=den[:])
                o_sb = smp.tile([G, HEAD_DIM], f32, tag="osb")
                nc.scalar.activation(out=o_sb[:], in_=o_ps[:], func=Copy, scale=rec[:, 0:1])
                nc.sync.dma_start(out=out[b], in_=o_sb[:])
    nc.compile()
    return nc


def kernel(q, k, v, k_cache, v_cache, block_tables, context_lens, slot_mapping):
    q = np.asarray(q, dtype=np.float32)
    k = np.asarray(k, dtype=np.float32)
    v = np.asarray(v, dtype=np.float32)
    kc = np.array(k_cache, dtype=np.float32).reshape(-1, NUM_KV_HEADS, HEAD_DIM)
    vcf = np.array(v_cache, dtype=np.float32).reshape(-1, NUM_KV_HEADS, HEAD_DIM)
    bt = np.clip(np.asarray(block_tables, dtype=np.int64), 0, NUM_BLOCKS - 1)
    cl = np.asarray(context_lens, dtype=np.int64)
    sm = np.asarray(slot_mapping, dtype=np.int64)

    # current-step K/V scatter (reference._store_kv), host-side while staging
    valid = sm >= 0
    kc[sm[valid]] = k[valid]
    vcf[sm[valid]] = v[valid]
    kc = kc.reshape(NUM_BLOCKS, BLOCK_SIZE, NUM_KV_HEADS, HEAD_DIM)
    vcf = vcf.reshape(NUM_BLOCKS, BLOCK_SIZE, NUM_KV_HEADS, HEAD_DIM)

    NB = np.maximum(1, -(-cl // BLOCK_SIZE)).astype(np.int64)

    # additive boundary mask, permuted token order (col c of a block holds
    # token 2*(c%128) + c//128), replicated across the G query heads
    c = np.arange(BLOCK_SIZE)
    tok = 2 * (c % P) + (c // P)
    mask = np.zeros((BATCH, G, BLOCK_SIZE), dtype=np.float32)
    for b in range(BATCH):
        cl_loc = cl[b] - BLOCK_SIZE * (NB[b] - 1)
        mask[b, :, :] = np.where(tok < cl_loc, 0.0, -1e9)[None, :]
    mask = np.ascontiguousarray(mask.transpose(1, 0, 2).reshape(G, BATCH * BLOCK_SIZE))

    key = (bt.tobytes(), NB.tobytes(), cl.tobytes())
    nc = _nc_cache.get(key)
    if nc is None:
        nc = _build_nc([int(x) for x in NB], [[int(x) for x in row] for row in bt])
        _nc_cache.clear()
        _nc_cache[key] = nc

    qg = q.reshape(BATCH, NUM_KV_HEADS, G, HEAD_DIM)
    in_maps = []
    for h in range(N_CORES):
        qt_h = np.ascontiguousarray(
            qg[:, h].transpose(2, 0, 1).reshape(P, BATCH * G)
        ).astype(bfloat16)
        in_maps.append(
            {
                "kc": np.ascontiguousarray(kc[:, :, h, :]),
                "vc": np.ascontiguousarray(vcf[:, :, h, :]),
                "qt": qt_h,
                "mask": mask,
            }
        )

    global _last_in_maps
    _last_in_maps = in_maps
    res = bass_utils.run_bass_kernel_spmd(nc, in_maps, core_ids=list(range(N_CORES)))
    outs = np.stack([res.results[h]["out"] for h in range(N_CORES)], axis=1)
    return np.ascontiguousarray(outs.reshape(BATCH, NUM_HEADS, HEAD_DIM)).astype(np.float32)
